# revision 1
# baseline (speedup 1.0000x reference)
"""Trainium2 Bass kernel for nn_BaseConvFFF (soft-routed conv mixture-of-experts).

Sharding: expert-parallel — each of the 8 cores computes 2 of the 16 leaves
(full batch), plus the full routing scores; host sums the 8 partial
mixture-weighted outputs.

Per-core device program:
  conv1 (3->64ch, 5x5 SAME) as one K=75 im2col matmul per 512-px tile
  routing convs (4 filters) ride the same im2col
  2x2 maxpool + relu fused into PSUM eviction (DVE), written into padded
  per-leaf planes with a +1-shifted copy in partitions 64:127 (K-pair packing)
  conv2 (64->64ch, 5x5 SAME) as 10 K=128 pair + 5 K=64 single matmuls per tile
  global spatial max (DVE reduce) -> 2-layer MLP (matmuls) -> mixture weighting
"""

import sys

if "/opt/trn_rl_repo" not in sys.path:
    sys.path.append("/opt/trn_rl_repo")

import numpy as np

B, CIN = 32, 3
NCORES = 8
HP = 36  # padded pooled plane (32 + 2*2)
HPROWS = 37  # +1 guard row for the shifted upper half
HPAD = 68  # padded conv1 input plane (64 + 2*2)
XPLANE = HPAD * HPAD  # 4624
XPADF = B * XPLANE + 64  # flat padded planes per channel + overrun tail
IMW = 64 * HPAD  # 4352: one im2col row (64 rows x 68, contiguous source)
OUT_W = 100

_cache = {}


def _build(opts=None):
    import concourse.bass as bass
    import concourse.tile as tile
    from concourse import bacc, mybir

    f32 = mybir.dt.float32
    f32r = mybir.dt.float32r
    MAX = mybir.AluOpType.max
    MULT = mybir.AluOpType.mult
    ADD = mybir.AluOpType.add
    AX = mybir.AxisListType.X
    ts = bass.ts

    o = dict(pool1=True, psc=2, psr=1, psd=2, imcol=3, tmpb=3, rtsb=2, slots=2, hq=False, skiprt=False, skipc2=False, skippool=False, skiprtp=False, ev='mix', rtcol=False, psr2=True, psd2=False)
    if opts:
        o.update(opts)
    nc = bacc.Bacc("TRN2", target_bir_lowering=False, debug=False, num_devices=NCORES)

    def din(name, shape, dt):
        return nc.dram_tensor(name, list(shape), dt, kind="ExternalInput").ap()

    xpadf = din("xpadf", (CIN, XPADF), f32r)
    w1T = din("w1T", (75, 128), f32r)
    rw = din("rw", (75, 4), f32r)
    cw2p = din("cw2p", (128, 2, 5, 2, 64), f32r)
    cw2s1 = din("cw2s1", (64, 2, 5, 64), f32r)
    cw2q = din("cw2q", (128, 2, 2, 64), f32r)
    w1sT = din("w1sT", (64, 2, 128), f32r)
    w2sT = din("w2sT", (128, 2, 100), f32r)
    b1sT = din("b1sT", (128, 2), f32)
    b2p = din("b2p", (2, 100), f32r)
    rbias = din("rbias", (4, 1), f32)
    alf = din("alf", (4, 1), f32)
    bet = din("bet", (4, 1), f32)
    hpz = din("hpz", (128, 4, HPROWS, HP), f32r)
    out = nc.dram_tensor("out", [B, OUT_W], f32, kind="ExternalOutput").ap()

    with tile.TileContext(nc) as tc:
        with (
            tc.tile_pool(name="const", bufs=1) as cp,
            tc.tile_pool(name="pers", bufs=1) as pers,
        ):
            w1T_t = cp.tile([75, 128], f32r)
            nc.sync.dma_start(w1T_t[:], w1T)
            rw_t = cp.tile([75, 4], f32r)
            nc.sync.dma_start(rw_t[:], rw)
            cw2p_t = cp.tile([128, 2, 5, 2, 64], f32r)
            nc.sync.dma_start(cw2p_t[:], cw2p)
            cw2s1_t = cp.tile([64, 2, 5, 64], f32r)
            nc.sync.dma_start(cw2s1_t[:], cw2s1)
            cw2q_t = cp.tile([128, 2, 2, 64], f32r)
            nc.sync.dma_start(cw2q_t[:], cw2q)
            w1sT_t = cp.tile([64, 2, 128], f32r)
            nc.sync.dma_start(w1sT_t[:], w1sT)
            w2sT_t = cp.tile([128, 2, 100], f32r)
            nc.sync.dma_start(w2sT_t[:], w2sT)
            b1sT_t = cp.tile([128, 2], f32)
            nc.sync.dma_start(b1sT_t[:], b1sT)
            b2p_t = cp.tile([2, 100], f32r)
            nc.sync.dma_start(b2p_t[:], b2p)
            rbias_t = cp.tile([4, 1], f32)
            nc.sync.dma_start(rbias_t[:], rbias)
            alf_t = cp.tile([4, 1], f32)
            nc.sync.dma_start(alf_t[:], alf)
            bet_t = cp.tile([4, 1], f32)
            nc.sync.dma_start(bet_t[:], bet)

            # persistent working buffers
            hp0 = pers.tile([128, o["slots"], HPROWS, HP], f32r)
            hp1 = pers.tile([128, o["slots"], HPROWS, HP], f32r)
            nc.sync.dma_start(hp0[:], hpz[:, : o["slots"]])
            nc.sync.dma_start(hp1[:], hpz[:, : o["slots"]])
            hps = (hp0, hp1)
            hq0 = pers.tile([128, o["slots"], HPROWS, HP], f32r)
            hq1 = pers.tile([128, o["slots"], HPROWS, HP], f32r)
            nc.sync.dma_start(hq0[:], hpz[:, : o["slots"]])
            nc.sync.dma_start(hq1[:], hpz[:, : o["slots"]])
            hqs = (hq0, hq1)
            featsc0 = pers.tile([64, 2 * B], f32)
            featsc1 = pers.tile([64, 2 * B], f32)
            featscs = (featsc0, featsc1)
            rtsc = pers.tile([128, B], f32)

            with (
                tc.tile_pool(name="imcol", bufs=o["imcol"]) as impool,
                tc.tile_pool(name="rts", bufs=o["rtsb"]) as rtspool,
                tc.tile_pool(name="tmp", bufs=o["tmpb"]) as tmppool,
                tc.tile_pool(name="rtp", bufs=2) as rtppool,
                tc.tile_pool(name="psc", bufs=o["psc"], space="PSUM") as pscp,
                tc.tile_pool(name="psr", bufs=o["psr"], space="PSUM") as psrp,
                tc.tile_pool(name="psd", bufs=o["psd"], space="PSUM") as psdp,
            ):
                for b in range(B):
                    slot = b % o["slots"]
                    # ---- im2col: partition p=(c,dy,dx) holds the padded
                    # plane shifted by (dy,dx) — contiguous 4352-elem source
                    imc = impool.tile([75, IMW], f32r)
                    for c in range(CIN):
                        src = bass.AP(
                            xpadf.tensor,
                            c * XPADF + b * XPLANE,
                            [[HPAD, 5], [1, 5], [1, IMW]],
                        )
                        nc.sync.dma_start(imc[c * 25 : (c + 1) * 25, :], src)
                    imcv = imc.rearrange("p (y x) -> p y x", y=64, x=HPAD)

                    rts = rtspool.tile([4, 4096], f32)
                    rtsv = rts
                    for pair in range(4):
                        if o["psr2"]:
                            psR2 = psrp.tile([4, 2, 512], f32, name="psR2")
                        # conv1: 128 out-ch (2 leaves), K=75 matmuls into a
                        # 2-bank psum tile (two 512-px halves)
                        psC2 = pscp.tile([128, 2, 512], f32)
                        for h in range(2):
                            t = 2 * pair + h
                            rhs = imcv[:, 8 * t : 8 * t + 8, 0:64]
                            nc.tensor.matmul(
                                psC2[:, h, :], w1T_t[:], rhs,
                                start=True, stop=True,
                            )
                            # routing: 4 filters, same rhs
                            if o["skiprt"]:
                                pass
                            elif o["psr2"]:
                                nc.tensor.matmul(
                                    psR2[:, h, :], rw_t[:], rhs,
                                    start=True, stop=True,
                                )
                            elif o["rtcol"]:
                                psRb = psrp.tile([128, 512], f32, name="psRb")
                                psRv = psRb[64:68, :]
                                nc.tensor.matmul(
                                    psRv, rw_t[:], rhs,
                                    start=True, stop=True, tile_position=(0, 64),
                                )
                            else:
                                psR = psrp.tile([4, 512], f32)
                                psRv = psR[:]
                                nc.tensor.matmul(
                                    psRv, rw_t[:], rhs,
                                    start=True, stop=True,
                                )
                            if not o["skiprt"] and not o["psr2"]:
                                nc.scalar.activation(
                                    rts[:, ts(t, 512)], psRv,
                                    mybir.ActivationFunctionType.Copy,
                                )
                        if not o["skiprt"] and o["psr2"]:
                            nc.scalar.activation(
                                rts[:, ts(pair, 1024)], psR2[:],
                                mybir.ActivationFunctionType.Copy,
                            )
                        # maxpool 2x2 + relu eviction (both halves at once)
                        y0 = 2 + 8 * pair
                        if o["skippool"]:
                            pass
                        elif o["pool1"]:
                            # one 5D reduce does the whole 2x2 pool
                            pcv = psC2.rearrange(
                                "p h (yb wy x wx) -> p (h yb) x wy wx",
                                yb=4, wy=2, x=32, wx=2,
                            )
                            tx = tmppool.tile([128, 8, 32], f32)
                            nc.vector.tensor_reduce(
                                tx[:], pcv[:], axis=mybir.AxisListType.XY, op=MAX
                            )
                            for leaf in range(2):
                                hp = hps[leaf]
                                th = tx[64 * leaf : 64 * leaf + 64, :, :]
                                if o["ev"] == "act":
                                    nc.scalar.activation(
                                        hp[0:64, slot, y0 : y0 + 8, 2:34], th,
                                        mybir.ActivationFunctionType.Relu,
                                    )
                                else:
                                    wr = (
                                        nc.gpsimd if o["ev"] == "gp" else nc.vector
                                    )
                                    wr.tensor_scalar_max(
                                        hp[0:64, slot, y0 : y0 + 8, 2:34], th, 0.0
                                    )
                                if o["ev"] in ("gp", "dve4"):
                                    wr.tensor_scalar_max(
                                        hp[64:128, slot, y0 : y0 + 8, 1:33], th, 0.0
                                    )
                                elif o["ev"] in ("mix", "act"):
                                    nc.scalar.activation(
                                        hp[64:128, slot, y0 : y0 + 8, 1:33], th,
                                        mybir.ActivationFunctionType.Relu,
                                    )
                        else:
                            pcv = psC2.rearrange(
                                "p h (y x t) -> p (h y) x t", y=8, x=32, t=2
                            )
                            tx = tmppool.tile([128, 8, 2, 32], f32)
                            txv = tx.rearrange("p a b x -> p (a b) x")
                            nc.vector.reduce_max(txv[:], pcv[:], axis=AX)
                            for leaf in range(2):
                                hp = hps[leaf]
                                dst = hp[0:64, slot, y0 : y0 + 8, 2:34]
                                nc.vector.scalar_tensor_tensor(
                                    dst,
                                    tx[64 * leaf : 64 * leaf + 64, :, 0, :],
                                    0.0,
                                    tx[64 * leaf : 64 * leaf + 64, :, 1, :],
                                    op0=MAX, op1=MAX,
                                )
                                # shifted (+1 elem) copy for conv2 K-pair packing
                                nc.gpsimd.tensor_copy(
                                    hp[64:128, slot, y0 : y0 + 8, 1:33], dst
                                )

                    if o["ev"] == "dma":
                        for leaf in range(2):
                            hp = hps[leaf]
                            nc.sync.dma_start(
                                hp[64:128, slot, 2:34, 1:33],
                                hp[0:64, slot, 2:34, 2:34],
                            )
                    if o["skiprt"]:
                        nc.vector.memset(rts[:], 0.0)
                    if o["hq"]:
                        # hq: lower = relu'd pooled plane, upper = same shifted
                        # one row up (+36) -> dy-pairs for the dx=4 taps
                        for leaf in range(2):
                            hp, hq = hps[leaf], hqs[leaf]
                            nc.sync.dma_start(
                                hq[0:64, slot, 2:34, 2:34],
                                hp[0:64, slot, 2:34, 2:34],
                            )
                            nc.sync.dma_start(
                                hq[64:128, slot, 1:33, 2:34],
                                hp[0:64, slot, 2:34, 2:34],
                            )

                    # routing per-image: repartition [4,4096] -> [(d j), 128]
                    if not o["skiprtp"]:
                        rtp = rtppool.tile([128, 128], f32)
                        nc.sync.dma_start(rtp[:], rts.rearrange("d (j e) -> d j e", j=32))
                        nc.vector.reduce_max(rtsc[:, b : b + 1], rtp[:], axis=AX)

                    # ---- conv2 per leaf: 10 K=128 dx-pairs + dx=4 taps
                    for leaf in range(2) if not o["skipc2"] else []:
                        hp = hps[leaf]
                        if o["psd2"]:
                            psD2 = psdp.tile([64, 2, 512], f32, name="psD2")
                        for t2 in range(2):
                            psD = psD2[:, t2, :] if o["psd2"] else psdp.tile([64, 512], f32)
                            first = True
                            for dy in range(5):
                                for j in range(2):
                                    nc.tensor.matmul(
                                        psD[:],
                                        cw2p_t[:, leaf, dy, j, :],
                                        hp[:, slot, 16 * t2 + dy : 16 * t2 + dy + 16,
                                           2 * j : 2 * j + 32],
                                        start=first, stop=False,
                                    )
                                    first = False
                                if not o["hq"]:
                                    nc.tensor.matmul(
                                        psD[:],
                                        cw2s1_t[:, leaf, dy, :],
                                        hp[0:64, slot, 16 * t2 + dy : 16 * t2 + dy + 16,
                                           4:36],
                                        start=False, stop=(dy == 4),
                                    )
                            if o["hq"]:
                                hq = hqs[leaf]
                                for q in range(2):
                                    nc.tensor.matmul(
                                        psD[:],
                                        cw2q_t[:, leaf, q, :],
                                        hq[:, slot,
                                           16 * t2 + 2 * q : 16 * t2 + 2 * q + 16,
                                           4:36],
                                        start=False, stop=False,
                                    )
                                nc.tensor.matmul(
                                    psD[:],
                                    cw2s1_t[:, leaf, 4, :],
                                    hp[0:64, slot, 16 * t2 + 4 : 16 * t2 + 4 + 16,
                                       4:36],
                                    start=False, stop=True,
                                )
                            if not o["psd2"]:
                                nc.vector.reduce_max(
                                    featscs[leaf][:, 2 * b + t2 : 2 * b + t2 + 1],
                                    psD[:], axis=AX,
                                )
                        if o["psd2"]:
                            nc.vector.reduce_max(
                                featscs[leaf][:, b : b + 1],
                                psD2[:].rearrange("p q n -> p (q n)"), axis=AX,
                            )

            # ---------------- finalize: routing mix + MLP ----------------
            with (
                tc.tile_pool(name="fin", bufs=1) as fin,
                tc.tile_pool(name="psm", bufs=1, space="PSUM") as psm,
            ):
                rtj = fin.tile([4, 32, B], f32)
                nc.sync.dma_start(rtj[:], rtsc[:])
                scoresT = fin.tile([4, B], f32)
                nc.vector.reduce_max(
                    scoresT[:], rtj.rearrange("d j b -> d b j"), axis=AX
                )
                sg = fin.tile([4, B], f32)
                nc.scalar.activation(
                    sg[:], scoresT[:], mybir.ActivationFunctionType.Sigmoid,
                    bias=rbias_t[:, 0:1],
                )
                fsel = fin.tile([4, B], f32)
                nc.vector.tensor_scalar(
                    fsel[:], sg[:], alf_t[:, 0:1], bet_t[:, 0:1], op0=MULT, op1=ADD
                )
                fT = fin.tile([B, 4], f32)
                for d in range(4):
                    nc.sync.dma_start(fT[:, d : d + 1], fsel[d : d + 1, :])
                t01 = fin.tile([B, 1], f32)
                nc.vector.tensor_mul(t01[:], fT[:, 0:1], fT[:, 1:2])
                m012 = fin.tile([B, 1], f32)
                nc.vector.tensor_mul(m012[:], t01[:], fT[:, 2:3])
                mixpair = fin.tile([B, 2], f32)
                nc.vector.tensor_mul(mixpair[:, 1:2], m012[:], fT[:, 3:4])
                nc.vector.tensor_sub(mixpair[:, 0:1], m012[:], mixpair[:, 1:2])
                mixpR = fin.tile([B, 2], f32r)
                nc.vector.tensor_copy(mixpR[:], mixpair[:])
                mixT = fin.tile([2, B], f32r)
                for leaf in range(2):
                    nc.sync.dma_start(
                        mixT[leaf : leaf + 1, :], mixpR[:, leaf : leaf + 1]
                    )

                if o["skipc2"]:
                    nc.vector.memset(featsc0[:], 0.0)
                    nc.vector.memset(featsc1[:], 0.0)
                if o["skiprtp"]:
                    nc.vector.memset(rtsc[:], 0.0)
                ps2s = []
                for leaf in range(2):
                    featT = fin.tile([64, B], f32r, name=f"featT{leaf}")
                    if o["psd2"]:
                        nc.vector.tensor_scalar_max(
                            featT[:], featscs[leaf][:, 0:B], 0.0
                        )
                    else:
                        nc.vector.reduce_max(
                            featT[:],
                            featscs[leaf].rearrange("p (b t) -> p b t", t=2),
                            axis=AX,
                        )
                        nc.vector.tensor_scalar_max(featT[:], featT[:], 0.0)
                    ps1 = psm.tile([128, B], f32, name=f"ps1_{leaf}")
                    nc.tensor.matmul(
                        ps1[:], w1sT_t[:, leaf, :], featT[:], start=True, stop=True
                    )
                    h1b = fin.tile([128, B], f32r, name=f"h1b{leaf}")
                    nc.vector.tensor_scalar_add(
                        h1b[:], ps1[:], b1sT_t[:, leaf : leaf + 1]
                    )
                    ps2 = psm.tile([B, OUT_W], f32, name=f"ps2_{leaf}")
                    nc.tensor.matmul(
                        ps2[:], h1b[:], w2sT_t[:, leaf, :], start=True, stop=True
                    )
                    ps2s.append(ps2)

                psb = psm.tile([B, OUT_W], f32)
                nc.tensor.matmul(psb[:], mixT[:], b2p_t[:], start=True, stop=True)

                acc = fin.tile([B, OUT_W], f32)
                nc.vector.tensor_scalar_mul(acc[:], ps2s[0][:], mixpair[:, 0:1])
                acc2 = fin.tile([B, OUT_W], f32)
                nc.vector.scalar_tensor_tensor(
                    acc2[:], ps2s[1][:], mixpair[:, 1:2], acc[:], op0=MULT, op1=ADD
                )
                osb = fin.tile([B, OUT_W], f32)
                nc.vector.tensor_add(osb[:], acc2[:], psb[:])
                nc.sync.dma_start(out, osb[:])

    nc.compile()
    return nc


def host_pack(inputs, core):
    x = np.ascontiguousarray(np.asarray(inputs["x"], np.float32))
    node_weights = np.asarray(inputs["node_weights"], np.float32)
    node_biases = np.asarray(inputs["node_biases"], np.float32)
    cw1s = np.asarray(inputs["cw1s"], np.float32)
    cw2s = np.asarray(inputs["cw2s"], np.float32)
    w1s = np.asarray(inputs["w1s"], np.float32)
    b1s = np.asarray(inputs["b1s"], np.float32)
    w2s = np.asarray(inputs["w2s"], np.float32)
    b2s = np.asarray(inputs["b2s"], np.float32)

    l0 = 2 * core
    xpad = np.zeros((CIN, B, HPAD, HPAD), np.float32)
    xpad[:, :, 2:66, 2:66] = x.transpose(1, 0, 2, 3)
    xpadf = np.zeros((CIN, XPADF), np.float32)
    xpadf[:, : B * XPLANE] = xpad.reshape(CIN, -1)

    # conv1 lhsT (75, 128): row p=(c,dy,dx), col m=(leaf, ch)
    # cw1s[l,ch,c,dy,dx] -> transpose to (c,dy,dx, ch) then reshape
    w1T = np.zeros((75, 128), np.float32)
    for leaf in range(2):
        w1T[:, 64 * leaf : 64 * leaf + 64] = (
            cw1s[l0 + leaf].transpose(1, 2, 3, 0).reshape(75, 64)
        )
    idx = [0, 2, 6, 14]
    rw = node_weights[idx, 0].transpose(1, 2, 3, 0).reshape(75, 4).copy()

    cw2p = np.zeros((128, 2, 5, 2, 64), np.float32)
    cw2s1 = np.zeros((64, 2, 5, 64), np.float32)
    for leaf in range(2):
        w = cw2s[l0 + leaf]  # (m=64, ci=64, dy, dx)
        for dy in range(5):
            for j in range(2):
                cw2p[0:64, leaf, dy, j, :] = w[:, :, dy, 2 * j].T
                cw2p[64:128, leaf, dy, j, :] = w[:, :, dy, 2 * j + 1].T
            cw2s1[:, leaf, dy, :] = w[:, :, dy, 4].T
    cw2q = np.zeros((128, 2, 2, 64), np.float32)
    for leaf in range(2):
        w = cw2s[l0 + leaf]
        for q in range(2):
            cw2q[0:64, leaf, q, :] = w[:, :, 2 * q, 4].T
            cw2q[64:128, leaf, q, :] = w[:, :, 2 * q + 1, 4].T

    w1sT = np.stack([w1s[l0], w1s[l0 + 1]], axis=1)  # (64, 2, 128)
    w2sT = np.stack([w2s[l0], w2s[l0 + 1]], axis=1)  # (128, 2, 100)
    b1sT = np.stack([b1s[l0], b1s[l0 + 1]], axis=1)  # (128, 2)
    b2p = np.stack([b2s[l0], b2s[l0 + 1]], axis=0)  # (2, 100)

    rbias = np.zeros((4, 1), np.float32)
    alfv = np.zeros((4, 1), np.float32)
    betv = np.zeros((4, 1), np.float32)
    for d in range(4):
        plat = 2**d - 1
        g = l0 >> (3 - d)
        j, s = g >> 1, g & 1
        rbias[d, 0] = node_biases[plat + j, 0]
        if d < 3:
            alfv[d, 0], betv[d, 0] = (1.0, 0.0) if s == 1 else (-1.0, 1.0)
        else:
            alfv[d, 0], betv[d, 0] = 1.0, 0.0
    return dict(
        xpadf=xpadf, w1T=w1T, rw=rw, cw2p=cw2p, cw2s1=cw2s1, cw2q=cw2q, w1sT=w1sT,
        w2sT=np.ascontiguousarray(w2sT), b1sT=np.ascontiguousarray(b1sT),
        b2p=np.ascontiguousarray(b2p), rbias=rbias, alf=alfv, bet=betv,
        hpz=np.zeros((128, 4, HPROWS, HP), np.float32),
    )


def kernel(**inputs):
    from concourse import bass_utils

    if "nc" not in _cache:
        _cache["nc"] = _build()
    nc = _cache["nc"]
    in_maps = [host_pack(inputs, c) for c in range(NCORES)]
    res = bass_utils.run_bass_kernel_spmd(nc, in_maps, core_ids=list(range(NCORES)))
    total = np.zeros((B, OUT_W), np.float32)
    for c in range(NCORES):
        total += res.results[c]["out"]
    return total



# revision 2
# speedup vs baseline: 1.3983x; 1.3983x over previous
"""Trainium2 Bass kernel for nn_BaseConvFFF (soft-routed conv mixture-of-experts).

Sharding: expert-parallel — each of the 8 cores computes 2 of the 16 leaves
(full batch), plus the full routing scores; host sums the 8 partial
mixture-weighted outputs.

Per-core device program (per image):
  conv1 (3->64ch, 5x5 SAME): K=75 im2col matmul, M=128 (2 leaves x 64ch),
    8 passes of N=512
  routing convs (4 filters): M=128 = (32 out rows x 4 filters), K=108 =
    (3ch x 36 in rows), 5 dx passes of N=256 per image PAIR
  2x2 maxpool (DVE reduce) + relu eviction into per-leaf planes: lower
    half (Pool engine) plain, upper half (Act engine) shifted +1 ROW
  conv2 (64->64ch, 5x5 SAME): M=128 = (2 out-row parity x 64ch), K=128 =
    (in-row parity x 64ch), 15 passes (3 row-pairs x 5 dx) of N=512 per leaf
  global spatial max (DVE) -> 2-layer MLP -> mixture weighting
"""

import sys

if "/opt/trn_rl_repo" not in sys.path:
    sys.path.append("/opt/trn_rl_repo")

import numpy as np

B, CIN = 32, 3
NCORES = 8
HP = 36  # padded pooled plane cols (32 + 2*2)
HPROWS = 36  # padded pooled plane rows (must be even for the pair view)
HPAD = 68  # padded conv1 input plane (64 + 2*2)
XPLANE = HPAD * HPAD  # 4624
XPADF = B * XPLANE + 64  # flat padded planes per channel + overrun tail
IMW = 64 * HPAD  # 4352: one im2col row (64 rows x 68, contiguous source)
OUT_W = 100

_cache = {}


def _build(opts=None):
    import concourse.bass as bass
    import concourse.tile as tile
    from concourse import bacc, mybir

    f32 = mybir.dt.float32
    f32r = mybir.dt.float32r
    MAX = mybir.AluOpType.max
    MULT = mybir.AluOpType.mult
    ADD = mybir.AluOpType.add
    AX = mybir.AxisListType.X
    AXY = mybir.AxisListType.XY
    RELU = mybir.ActivationFunctionType.Relu

    o = dict(imcol=3, tmpb=3, xrb=2, psc=2, pse=2, psr=1, slots=2,
             lower_eng="gp", upper_eng="act")
    if opts:
        o.update(opts)
    nc = bacc.Bacc("TRN2", target_bir_lowering=False, debug=False, num_devices=NCORES)

    def din(name, shape, dt):
        return nc.dram_tensor(name, list(shape), dt, kind="ExternalInput").ap()

    xpadf = din("xpadf", (CIN, XPADF), f32r)
    w1T = din("w1T", (75, 128), f32r)
    rtw = din("rtw", (108, 5, 128), f32r)
    cw2n = din("cw2n", (128, 2, 3, 5, 128), f32r)
    w1sT = din("w1sT", (64, 2, 128), f32r)
    w2sT = din("w2sT", (128, 2, 100), f32r)
    b1sT = din("b1sT", (128, 2), f32)
    b2p = din("b2p", (2, 100), f32r)
    rbias = din("rbias", (4, 1), f32)
    alf = din("alf", (4, 1), f32)
    bet = din("bet", (4, 1), f32)
    hpz = din("hpz", (128, o["slots"], HPROWS, HP), f32r)
    out = nc.dram_tensor("out", [B, OUT_W], f32, kind="ExternalOutput").ap()

    with tile.TileContext(nc) as tc:
        with (
            tc.tile_pool(name="const", bufs=1) as cp,
            tc.tile_pool(name="pers", bufs=1) as pers,
        ):
            w1T_t = cp.tile([75, 128], f32r)
            nc.sync.dma_start(w1T_t[:], w1T)
            rtw_t = cp.tile([108, 5, 128], f32r)
            nc.sync.dma_start(rtw_t[:], rtw)
            cw2n_t = cp.tile([128, 2, 3, 5, 128], f32r)
            nc.sync.dma_start(cw2n_t[:], cw2n)
            w1sT_t = cp.tile([64, 2, 128], f32r)
            nc.sync.dma_start(w1sT_t[:], w1sT)
            w2sT_t = cp.tile([128, 2, 100], f32r)
            nc.sync.dma_start(w2sT_t[:], w2sT)
            b1sT_t = cp.tile([128, 2], f32)
            nc.sync.dma_start(b1sT_t[:], b1sT)
            b2p_t = cp.tile([2, 100], f32r)
            nc.sync.dma_start(b2p_t[:], b2p)
            rbias_t = cp.tile([4, 1], f32)
            nc.sync.dma_start(rbias_t[:], rbias)
            alf_t = cp.tile([4, 1], f32)
            nc.sync.dma_start(alf_t[:], alf)
            bet_t = cp.tile([4, 1], f32)
            nc.sync.dma_start(bet_t[:], bet)

            # persistent working buffers
            # hp tiles: partitions 0:64 = pooled plane (relu'd), partitions
            # 64:128 = same plane shifted UP one row (for conv2 dy pairing)
            hp0 = pers.tile([128, o["slots"], HPROWS, HP], f32r)
            hp1 = pers.tile([128, o["slots"], HPROWS, HP], f32r)
            nc.sync.dma_start(hp0[:], hpz)
            nc.sync.dma_start(hp1[:], hpz)
            hps = (hp0, hp1)
            featsc0 = pers.tile([128, B], f32)
            featsc1 = pers.tile([128, B], f32)
            featscs = (featsc0, featsc1)
            rtsc = pers.tile([128, B], f32)

            with (
                tc.tile_pool(name="imcol", bufs=o["imcol"]) as impool,
                tc.tile_pool(name="xr", bufs=o["xrb"]) as xrpool,
                tc.tile_pool(name="tmp", bufs=o["tmpb"]) as tmppool,
                tc.tile_pool(name="psc", bufs=o["psc"], space="PSUM") as pscp,
                tc.tile_pool(name="pse", bufs=o["pse"], space="PSUM") as psep,
                tc.tile_pool(name="psr", bufs=o["psr"], space="PSUM") as psrp,
            ):
                def emit_conv2(b):
                    slot = b % o["slots"]
                    for leaf in range(2):
                        hp = hps[leaf]
                        hpv = hp.rearrange("p s (r t) c -> p s r t c", t=2)
                        psE = psep.tile([128, 16, 32], f32, name="psE")
                        k = 0
                        for p in range(3):
                            for dx in range(5):
                                k += 1
                                nc.tensor.matmul(
                                    psE[:],
                                    cw2n_t[:, leaf, p, dx, :],
                                    hpv[:, slot, p : p + 16, 0, dx : dx + 32],
                                    start=(k == 1), stop=(k == 15),
                                )
                        nc.vector.tensor_reduce(
                            featscs[leaf][:, b : b + 1],
                            psE.rearrange("p q x -> p (q x)"),
                            axis=AX, op=MAX,
                        )

                for b in range(B):
                    slot = b % o["slots"]
                    # ---- im2col: partition p=(c,dy,dx) holds the padded
                    # plane shifted by (dy,dx) — contiguous 4352-elem source
                    imc = impool.tile([75, IMW], f32r)
                    for c in range(CIN):
                        src = bass.AP(
                            xpadf.tensor,
                            c * XPADF + b * XPLANE,
                            [[HPAD, 5], [1, 5], [1, IMW]],
                        )
                        nc.sync.dma_start(imc[c * 25 : (c + 1) * 25, :], src)
                    imcv = imc.rearrange("p (y x) -> p y x", y=64, x=HPAD)

                    # ---- routing input rows (image pair): partition
                    # (ch, rr) holds padded rows 32h+rr of images b, b+1
                    if b % 2 == 0:
                        xr = xrpool.tile([108, 4, HPAD], f32r)
                        for c in range(CIN):
                            for i in range(2):
                                src = bass.AP(
                                    xpadf.tensor,
                                    c * XPADF + (b + i) * XPLANE,
                                    [[HPAD, 36], [32 * HPAD, 2], [1, HPAD]],
                                )
                                nc.sync.dma_start(
                                    xr[36 * c : 36 * c + 36, 2 * i : 2 * i + 2, :],
                                    src,
                                )

                    # ---- conv1: 128 out-ch (2 leaves), K=75 matmuls into a
                    # 2-bank psum tile (two 512-px halves)
                    for pair in range(4):
                        psC2 = pscp.tile([128, 2, 512], f32)
                        for h in range(2):
                            t = 2 * pair + h
                            rhs = imcv[:, 8 * t : 8 * t + 8, 0:64]
                            nc.tensor.matmul(
                                psC2[:, h, :], w1T_t[:], rhs,
                                start=True, stop=True,
                            )
                        # maxpool 2x2 via one 5D reduce (both halves at once)
                        pcv = psC2.rearrange(
                            "p h (yb wy x wx) -> p (h yb) x wy wx",
                            yb=4, wy=2, x=32, wx=2,
                        )
                        tx = tmppool.tile([128, 8, 32], f32)
                        nc.vector.tensor_reduce(tx[:], pcv[:], axis=AXY, op=MAX)
                        y0 = 2 + 8 * pair
                        for leaf in range(2):
                            hp = hps[leaf]
                            th = tx[64 * leaf : 64 * leaf + 64, :, :]
                            # lower block: relu'd pooled plane
                            if o["lower_eng"] == "gp":
                                nc.gpsimd.tensor_scalar_max(
                                    hp[0:64, slot, y0 : y0 + 8, 2:34], th, 0.0
                                )
                            else:
                                nc.vector.tensor_scalar_max(
                                    hp[0:64, slot, y0 : y0 + 8, 2:34], th, 0.0
                                )
                            # upper block: same, shifted one ROW up
                            if o["upper_eng"] == "act":
                                nc.scalar.activation(
                                    hp[64:128, slot, y0 - 1 : y0 + 7, 2:34], th,
                                    RELU,
                                )
                            else:
                                nc.gpsimd.tensor_scalar_max(
                                    hp[64:128, slot, y0 - 1 : y0 + 7, 2:34], th, 0.0
                                )

                    # ---- routing matmuls: M=(f,o), K=(ch,rr), accumulate
                    # over 5 dx passes; then max over (h, x) per image
                    if b % 2 == 0:
                        psR = psrp.tile([128, 4, 64], f32, name="psR")
                        for dx in range(5):
                            nc.tensor.matmul(
                                psR[:], rtw_t[:, dx, :], xr[:, :, dx : dx + 64],
                                start=(dx == 0), stop=(dx == 4),
                            )
                        nc.vector.tensor_reduce(
                            rtsc[:, b : b + 2],
                            psR.rearrange("m (i h) x -> m i (h x)", i=2),
                            axis=AX, op=MAX,
                        )

                    # ---- conv2 of the PREVIOUS image (keeps PE busy while
                    # this image's pool/evict chain completes)
                    if b > 0:
                        emit_conv2(b - 1)
                emit_conv2(B - 1)

            # ---------------- finalize: routing mix + MLP ----------------
            with (
                tc.tile_pool(name="fin", bufs=1) as fin,
                tc.tile_pool(name="psm", bufs=1, space="PSUM") as psm,
            ):
                rtj = fin.tile([4, 32, B], f32)
                nc.sync.dma_start(rtj[:], rtsc[:])
                scoresT = fin.tile([4, B], f32)
                nc.vector.reduce_max(
                    scoresT[:], rtj.rearrange("d j b -> d b j"), axis=AX
                )
                sg = fin.tile([4, B], f32)
                nc.scalar.activation(
                    sg[:], scoresT[:], mybir.ActivationFunctionType.Sigmoid,
                    bias=rbias_t[:, 0:1],
                )
                fsel = fin.tile([4, B], f32)
                nc.vector.tensor_scalar(
                    fsel[:], sg[:], alf_t[:, 0:1], bet_t[:, 0:1], op0=MULT, op1=ADD
                )
                fT = fin.tile([B, 4], f32)
                for d in range(4):
                    nc.sync.dma_start(fT[:, d : d + 1], fsel[d : d + 1, :])
                t01 = fin.tile([B, 1], f32)
                nc.vector.tensor_mul(t01[:], fT[:, 0:1], fT[:, 1:2])
                m012 = fin.tile([B, 1], f32)
                nc.vector.tensor_mul(m012[:], t01[:], fT[:, 2:3])
                mixpair = fin.tile([B, 2], f32)
                nc.vector.tensor_mul(mixpair[:, 1:2], m012[:], fT[:, 3:4])
                nc.vector.tensor_sub(mixpair[:, 0:1], m012[:], mixpair[:, 1:2])
                mixpR = fin.tile([B, 2], f32r)
                nc.vector.tensor_copy(mixpR[:], mixpair[:])
                mixT = fin.tile([2, B], f32r)
                for leaf in range(2):
                    nc.sync.dma_start(
                        mixT[leaf : leaf + 1, :], mixpR[:, leaf : leaf + 1]
                    )

                ps2s = []
                for leaf in range(2):
                    # fold the two out-row-parity halves of feat, with relu
                    fu = fin.tile([64, B], f32, name=f"fu{leaf}")
                    nc.sync.dma_start(fu[:], featscs[leaf][64:128, :])
                    featT = fin.tile([64, B], f32r, name=f"featT{leaf}")
                    nc.vector.scalar_tensor_tensor(
                        featT[:], featscs[leaf][0:64, :], 0.0, fu[:],
                        op0=MAX, op1=MAX,
                    )
                    ps1 = psm.tile([128, B], f32, name=f"ps1_{leaf}")
                    nc.tensor.matmul(
                        ps1[:], w1sT_t[:, leaf, :], featT[:], start=True, stop=True
                    )
                    h1b = fin.tile([128, B], f32r, name=f"h1b{leaf}")
                    nc.vector.tensor_scalar_add(
                        h1b[:], ps1[:], b1sT_t[:, leaf : leaf + 1]
                    )
                    ps2 = psm.tile([B, OUT_W], f32, name=f"ps2_{leaf}")
                    nc.tensor.matmul(
                        ps2[:], h1b[:], w2sT_t[:, leaf, :], start=True, stop=True
                    )
                    ps2s.append(ps2)

                psb = psm.tile([B, OUT_W], f32)
                nc.tensor.matmul(psb[:], mixT[:], b2p_t[:], start=True, stop=True)

                acc = fin.tile([B, OUT_W], f32)
                nc.vector.tensor_scalar_mul(acc[:], ps2s[0][:], mixpair[:, 0:1])
                acc2 = fin.tile([B, OUT_W], f32)
                nc.vector.scalar_tensor_tensor(
                    acc2[:], ps2s[1][:], mixpair[:, 1:2], acc[:], op0=MULT, op1=ADD
                )
                osb = fin.tile([B, OUT_W], f32)
                nc.vector.tensor_add(osb[:], acc2[:], psb[:])
                nc.sync.dma_start(out, osb[:])

    nc.compile()
    return nc


def host_pack(inputs, core, slots=2):
    x = np.ascontiguousarray(np.asarray(inputs["x"], np.float32))
    node_weights = np.asarray(inputs["node_weights"], np.float32)
    node_biases = np.asarray(inputs["node_biases"], np.float32)
    cw1s = np.asarray(inputs["cw1s"], np.float32)
    cw2s = np.asarray(inputs["cw2s"], np.float32)
    w1s = np.asarray(inputs["w1s"], np.float32)
    b1s = np.asarray(inputs["b1s"], np.float32)
    w2s = np.asarray(inputs["w2s"], np.float32)
    b2s = np.asarray(inputs["b2s"], np.float32)

    l0 = 2 * core
    xpad = np.zeros((CIN, B, HPAD, HPAD), np.float32)
    xpad[:, :, 2:66, 2:66] = x.transpose(1, 0, 2, 3)
    xpadf = np.zeros((CIN, XPADF), np.float32)
    xpadf[:, : B * XPLANE] = xpad.reshape(CIN, -1)

    # conv1 lhsT (75, 128): row p=(c,dy,dx), col m=(leaf, ch)
    w1T = np.zeros((75, 128), np.float32)
    for leaf in range(2):
        w1T[:, 64 * leaf : 64 * leaf + 64] = (
            cw1s[l0 + leaf].transpose(1, 2, 3, 0).reshape(75, 64)
        )

    # routing lhsT (108, 5, 128): row k=(ch, rr), col m=(f, o); per dx
    nw = node_weights[[0, 2, 6, 14], 0]  # (4, 3, 5, 5)
    rtw = np.zeros((108, 5, 128), np.float32)
    ovec = np.arange(32)
    for f in range(4):
        for ch in range(CIN):
            for dy in range(5):
                rtw[36 * ch + ovec + dy, :, 32 * f + ovec] = nw[f, ch, dy, :]

    # conv2 lhsT (128, 2, 3, 5, 128): row k=(r, ci), col m=(o, co);
    # block (r, o) of pass p holds W[:, :, dy=2p+r-o, dx] when 0<=dy<=4
    cw2n = np.zeros((128, 2, 3, 5, 128), np.float32)
    for leaf in range(2):
        w = cw2s[l0 + leaf]  # (co, ci, dy, dx)
        for p in range(3):
            for r in range(2):
                for oo in range(2):
                    dy = 2 * p + r - oo
                    if 0 <= dy <= 4:
                        cw2n[64 * r : 64 * r + 64, leaf, p, :, 64 * oo : 64 * oo + 64] = (
                            w[:, :, dy, :].transpose(1, 2, 0)
                        )

    w1sT = np.stack([w1s[l0], w1s[l0 + 1]], axis=1)  # (64, 2, 128)
    w2sT = np.stack([w2s[l0], w2s[l0 + 1]], axis=1)  # (128, 2, 100)
    b1sT = np.stack([b1s[l0], b1s[l0 + 1]], axis=1)  # (128, 2)
    b2p = np.stack([b2s[l0], b2s[l0 + 1]], axis=0)  # (2, 100)

    rbias = np.zeros((4, 1), np.float32)
    alfv = np.zeros((4, 1), np.float32)
    betv = np.zeros((4, 1), np.float32)
    for d in range(4):
        plat = 2**d - 1
        g = l0 >> (3 - d)
        j, s = g >> 1, g & 1
        rbias[d, 0] = node_biases[plat + j, 0]
        if d < 3:
            alfv[d, 0], betv[d, 0] = (1.0, 0.0) if s == 1 else (-1.0, 1.0)
        else:
            alfv[d, 0], betv[d, 0] = 1.0, 0.0
    return dict(
        xpadf=xpadf, w1T=w1T, rtw=rtw, cw2n=cw2n, w1sT=w1sT,
        w2sT=np.ascontiguousarray(w2sT), b1sT=np.ascontiguousarray(b1sT),
        b2p=np.ascontiguousarray(b2p), rbias=rbias, alf=alfv, bet=betv,
        hpz=np.zeros((128, slots, HPROWS, HP), np.float32),
    )


def kernel(**inputs):
    from concourse import bass_utils

    if "nc" not in _cache:
        _cache["nc"] = _build()
    nc = _cache["nc"]
    in_maps = [host_pack(inputs, c) for c in range(NCORES)]
    res = bass_utils.run_bass_kernel_spmd(nc, in_maps, core_ids=list(range(NCORES)))
    total = np.zeros((B, OUT_W), np.float32)
    for c in range(NCORES):
        total += res.results[c]["out"]
    return total


# revision 24
# speedup vs baseline: 2.0019x; 1.4317x over previous
"""Trainium2 Bass kernel for nn_BaseConvFFF (soft-routed conv mixture-of-experts).

Sharding: expert-parallel — each of the 8 cores computes 2 of the 16 leaves
(full batch), plus the full routing scores; host sums the 8 partial
mixture-weighted outputs.

Per-core device program (per image):
  conv1 (3->64ch, 5x5 SAME): K=75 im2col matmul, M=128 (2 leaves x 64ch),
    8 passes of N=512
  routing convs (4 filters): M=128 = (32 out rows x 4 filters), K=108 =
    (3ch x 36 in rows), 5 dx passes of N=256 per image PAIR
  2x2 maxpool (DVE reduce) + relu eviction into per-leaf planes: lower
    half (Pool engine) plain, upper half (Act engine) shifted +1 ROW
  conv2 (64->64ch, 5x5 SAME): M=128 = (2 out-row parity x 64ch), K=128 =
    (in-row parity x 64ch), 15 passes (3 row-pairs x 5 dx) of N=512 per leaf
  global spatial max (DVE) -> 2-layer MLP -> mixture weighting
"""

import sys

if "/opt/trn_rl_repo" not in sys.path:
    sys.path.append("/opt/trn_rl_repo")

import numpy as np

B, CIN = 32, 3
NCORES = 8
HP = 36  # padded pooled plane cols (32 + 2*2)
HPROWS = 36  # padded pooled plane rows (must be even for the pair view)
HPAD = 68  # padded conv1 input plane (64 + 2*2)
XPLANE = HPAD * HPAD  # 4624
XPADF = B * XPLANE + 64  # flat padded planes per channel + overrun tail
IMW = 64 * HPAD  # 4352: one im2col row (64 rows x 68, contiguous source)
OUT_W = 100

_cache = {}


def _build(opts=None):
    import concourse.bass as bass
    import concourse.tile as tile
    from concourse import bacc, mybir

    f32 = mybir.dt.float32
    f32r = mybir.dt.float32r
    MAX = mybir.AluOpType.max
    MULT = mybir.AluOpType.mult
    ADD = mybir.AluOpType.add
    AX = mybir.AxisListType.X
    AXY = mybir.AxisListType.XY
    RELU = mybir.ActivationFunctionType.Relu

    o = dict(imcol=5, tmpb=3, xrb=3, psc=2, pse=2, psr=2, slots=2,
             lower_eng="gp", upper_eng="act", hiprio=True, dbg=False)
    if opts:
        o.update(opts)
    nc = bacc.Bacc("TRN2", target_bir_lowering=False, debug=False, num_devices=NCORES)

    def din(name, shape, dt):
        return nc.dram_tensor(name, list(shape), dt, kind="ExternalInput").ap()

    imcd = din("imcd", (75, B, IMW), f32r)
    xrd = din("xrd", (108, B // 2, 4, HPAD), f32r)
    w1T = din("w1T", (75, 128), f32r)
    rtw = din("rtw", (108, 5, 128), f32r)
    cw2n = din("cw2n", (128, 2, 3, 5, 128), f32r)
    w1sT = din("w1sT", (64, 2, 128), f32r)
    w2sT = din("w2sT", (128, 2, 100), f32r)
    b1sT = din("b1sT", (128, 2), f32)
    b2p = din("b2p", (2, 100), f32r)
    rbias = din("rbias", (4, 1), f32)
    alf = din("alf", (4, 1), f32)
    bet = din("bet", (4, 1), f32)
    hpz = din("hpz", (128, HPROWS, HP), f32r)
    out = nc.dram_tensor("out", [B, OUT_W], f32, kind="ExternalOutput").ap()
    if o["dbg"]:
        dfeat0 = nc.dram_tensor("dfeat0", [128, B], f32, kind="ExternalOutput").ap()
        dfeat1 = nc.dram_tensor("dfeat1", [128, B], f32, kind="ExternalOutput").ap()
        drtsc = nc.dram_tensor("drtsc", [128, B], f32, kind="ExternalOutput").ap()
        dhp00 = nc.dram_tensor("dhp00", [128, HPROWS, HP], f32r, kind="ExternalOutput").ap()
        dhp10 = nc.dram_tensor("dhp10", [128, HPROWS, HP], f32r, kind="ExternalOutput").ap()

    with tile.TileContext(nc) as tc:
        with (
            tc.tile_pool(name="const", bufs=1) as cp,
            tc.tile_pool(name="pers", bufs=1) as pers,
        ):
            # critical consts first (needed by image 0's conv1/routing)
            w1T_t = cp.tile([75, 128], f32r)
            nc.sync.dma_start(w1T_t[:], w1T)
            rtw_t = cp.tile([108, 5, 128], f32r)
            nc.sync.dma_start(rtw_t[:], rtw)
            # big/late consts: allocate now, DMA after image 0's input DMAs
            cw2n_t = cp.tile([128, 2, 3, 5, 128], f32r)
            w1sT_t = cp.tile([64, 2, 128], f32r)
            w2sT_t = cp.tile([128, 2, 100], f32r)
            b1sT_t = cp.tile([128, 2], f32)
            b2p_t = cp.tile([2, 100], f32r)
            rbias_t = cp.tile([4, 1], f32)
            alf_t = cp.tile([4, 1], f32)
            bet_t = cp.tile([4, 1], f32)

            def emit_late_consts():
                nc.sync.dma_start(cw2n_t[:], cw2n)
                nc.sync.dma_start(w1sT_t[:], w1sT)
                nc.sync.dma_start(w2sT_t[:], w2sT)
                nc.sync.dma_start(b1sT_t[:], b1sT)
                nc.sync.dma_start(b2p_t[:], b2p)
                nc.sync.dma_start(rbias_t[:], rbias)
                nc.sync.dma_start(alf_t[:], alf)
                nc.sync.dma_start(bet_t[:], bet)

            # persistent working buffers
            # hp tiles (one per leaf x slot): partitions 0:64 = pooled plane
            # (relu'd), partitions 64:128 = same plane shifted UP one row
            # (for conv2 dy pairing). Zeroed on-device (pads stay zero).
            hps = [
                [pers.tile([128, HPROWS, HP], f32r, name=f"hp{l}_{s}")
                 for s in range(o["slots"])]
                for l in range(2)
            ]
            for s in range(o["slots"]):
                nc.sync.dma_start(hps[0][s][:], hpz)
                nc.sync.dma_start(hps[1][s][:], hpz)
            featsc0 = pers.tile([128, B], f32)
            featsc1 = pers.tile([128, B], f32)
            featscs = (featsc0, featsc1)
            rtsc = pers.tile([128, B], f32)

            with (
                tc.tile_pool(name="imcol", bufs=o["imcol"]) as impool,
                tc.tile_pool(name="xr", bufs=o["xrb"]) as xrpool,
                tc.tile_pool(name="tmp", bufs=o["tmpb"]) as tmppool,
                tc.tile_pool(name="psc", bufs=o["psc"], space="PSUM") as pscp,
                tc.tile_pool(name="pse", bufs=o["pse"], space="PSUM") as psep,
                tc.tile_pool(name="psr", bufs=o["psr"], space="PSUM") as psrp,
            ):
                def conv2_stream(b):
                    """Yields after each conv2 matmul so the caller can
                    interleave chunks between conv1 pairs of the next image."""
                    slot = b % o["slots"]
                    for leaf in range(2):
                        hpv = hps[leaf][slot].rearrange(
                            "p (r t) c -> p r t c", t=2
                        )
                        psE = psep.tile([128, 16, 32], f32, name="psE")
                        k = 0
                        for p in range(3):
                            for dx in range(5):
                                k += 1
                                nc.tensor.matmul(
                                    psE[:],
                                    cw2n_t[:, leaf, p, dx, :],
                                    hpv[:, p : p + 16, 0, dx : dx + 32],
                                    start=(k == 1), stop=(k == 15),
                                )
                                yield
                        nc.vector.tensor_reduce(
                            featscs[leaf][:, b : b + 1],
                            psE.rearrange("p q x -> p (q x)"),
                            axis=AX, op=MAX,
                        )

                def pull(gen, n):
                    if gen is not None:
                        for _ in range(n):
                            if next(gen, "done") == "done":
                                break

                gen = None
                for b in range(B):
                    slot = b % o["slots"]
                    # ---- im2col: partition p=(c,dy,dx) holds the padded
                    # plane shifted by (dy,dx) — contiguous 4352-elem source
                    imc = impool.tile([75, IMW], f32r)
                    nc.sync.dma_start(imc[:], imcd[:, b, :])
                    imcv = imc.rearrange("p (y x) -> p y x", y=64, x=HPAD)

                    # ---- routing input rows (image pair): partition
                    # (ch, rr) holds padded rows 32h+rr of images b, b+1
                    if b % 2 == 0:
                        xr = xrpool.tile([108, 4, HPAD], f32r)
                        nc.sync.dma_start(xr[:], xrd[:, b // 2, :, :])
                    if b == 0:
                        emit_late_consts()

    # ---- conv1: 128 out-ch (2 leaves), K=75 matmuls into a
                    # 2-bank psum tile (two 512-px halves); conv2 chunks of
                    # the previous image fill PE while reduces/evicts run.
                    # The conv1->reduce->evict chain is emitted at high
                    # priority so the scheduler starts it as soon as inputs
                    # land; conv2 of the previous image then fills the PE
                    # while the chain completes.
                    import contextlib
                    hctx = tc.high_priority if o["hiprio"] else contextlib.nullcontext
                    for pair in range(4):
                      with hctx():
                        psC2 = pscp.tile([128, 2, 512], f32)
                        for h in range(2):
                            t = 2 * pair + h
                            rhs = imcv[:, 8 * t : 8 * t + 8, 0:64]
                            nc.tensor.matmul(
                                psC2[:, h, :], w1T_t[:], rhs,
                                start=True, stop=True,
                            )
                        # maxpool 2x2 via one 5D reduce (single PSUM input —
                        # the HW verifier allows only one PSUM read per op),
                        # then gpsimd relu-evict lower + Act relu-evict the
                        # +1-row shifted upper block.
                        pcv = psC2.rearrange(
                            "p h (yb wy x wx) -> p (h yb) x wy wx",
                            yb=4, wy=2, x=32, wx=2,
                        )
                        tx = tmppool.tile([128, 8, 32], f32)
                        nc.vector.tensor_reduce(tx[:], pcv[:], axis=AXY, op=MAX)
                        y0 = 2 + 8 * pair
                        for leaf in range(2):
                            hp = hps[leaf][slot]
                            th = tx[64 * leaf : 64 * leaf + 64, :, :]
                            nc.gpsimd.tensor_scalar_max(
                                hp[0:64, y0 : y0 + 8, 2:34], th, 0.0
                            )
                            # upper block: same, shifted one ROW up
                            nc.scalar.activation(
                                hp[64:128, y0 - 1 : y0 + 7, 2:34], th, RELU
                            )
                      if pair >= 1:
                          pull(gen, 8)

                    # ---- routing matmuls: M=(f,o), K=(ch,rr), accumulate
                    # over 5 dx passes; then max over (h, x) per image
                    if b % 2 == 0:
                        # full-bank tile: matmul start=True resets the whole
                        # 2KB PSUM bank, so psR must not share a bank
                        psR = psrp.tile([128, 2, 2, 128], f32, name="psR")
                        for dx in range(5):
                            nc.tensor.matmul(
                                psR[:, 0], rtw_t[:, dx, :], xr[:, :, dx : dx + 64],
                                start=(dx == 0), stop=(dx == 4),
                            )
                        nc.vector.tensor_reduce(
                            rtsc[:, b : b + 2], psR[:, 0], axis=AX, op=MAX,
                        )
                    pull(gen, 99)
                    gen = conv2_stream(b)
                pull(gen, 99)

            # ---------------- finalize: routing mix + MLP ----------------
            with (
                tc.tile_pool(name="fin", bufs=1) as fin,
                tc.tile_pool(name="psm", bufs=1, space="PSUM") as psm,
            ):
                if o["dbg"]:
                    nc.sync.dma_start(dfeat0, featsc0[:])
                    nc.sync.dma_start(dfeat1, featsc1[:])
                    nc.sync.dma_start(drtsc, rtsc[:])
                    nc.sync.dma_start(dhp00, hps[0][0][:])
                    nc.sync.dma_start(dhp10, hps[1][0][:])
                rtj = fin.tile([4, 32, B], f32)
                nc.sync.dma_start(rtj[:], rtsc[:])
                scoresT = fin.tile([4, B], f32)
                nc.vector.reduce_max(
                    scoresT[:], rtj.rearrange("d j b -> d b j"), axis=AX
                )
                sg = fin.tile([4, B], f32)
                nc.scalar.activation(
                    sg[:], scoresT[:], mybir.ActivationFunctionType.Sigmoid,
                    bias=rbias_t[:, 0:1],
                )
                fsel = fin.tile([4, B], f32)
                nc.vector.tensor_scalar(
                    fsel[:], sg[:], alf_t[:, 0:1], bet_t[:, 0:1], op0=MULT, op1=ADD
                )
                fT = fin.tile([B, 4], f32)
                for d in range(4):
                    nc.sync.dma_start(fT[:, d : d + 1], fsel[d : d + 1, :])
                t01 = fin.tile([B, 1], f32)
                nc.vector.tensor_mul(t01[:], fT[:, 0:1], fT[:, 1:2])
                m012 = fin.tile([B, 1], f32)
                nc.vector.tensor_mul(m012[:], t01[:], fT[:, 2:3])
                mixpair = fin.tile([B, 2], f32)
                nc.vector.tensor_mul(mixpair[:, 1:2], m012[:], fT[:, 3:4])
                nc.vector.tensor_sub(mixpair[:, 0:1], m012[:], mixpair[:, 1:2])
                mixpR = fin.tile([B, 2], f32r)
                nc.vector.tensor_copy(mixpR[:], mixpair[:])
                mixT = fin.tile([2, B], f32r)
                for leaf in range(2):
                    nc.sync.dma_start(
                        mixT[leaf : leaf + 1, :], mixpR[:, leaf : leaf + 1]
                    )

                ps2s = []
                for leaf in range(2):
                    # fold the two out-row-parity halves of feat, with relu
                    fu = fin.tile([64, B], f32, name=f"fu{leaf}")
                    nc.sync.dma_start(fu[:], featscs[leaf][64:128, :])
                    featT = fin.tile([64, B], f32r, name=f"featT{leaf}")
                    nc.vector.scalar_tensor_tensor(
                        featT[:], featscs[leaf][0:64, :], 0.0, fu[:],
                        op0=MAX, op1=MAX,
                    )
                    ps1 = psm.tile([128, B], f32, name=f"ps1_{leaf}")
                    nc.tensor.matmul(
                        ps1[:], w1sT_t[:, leaf, :], featT[:], start=True, stop=True
                    )
                    h1b = fin.tile([128, B], f32r, name=f"h1b{leaf}")
                    nc.vector.tensor_scalar_add(
                        h1b[:], ps1[:], b1sT_t[:, leaf : leaf + 1]
                    )
                    ps2 = psm.tile([B, OUT_W], f32, name=f"ps2_{leaf}")
                    nc.tensor.matmul(
                        ps2[:], h1b[:], w2sT_t[:, leaf, :], start=True, stop=True
                    )
                    ps2s.append(ps2)

                psb = psm.tile([B, OUT_W], f32)
                nc.tensor.matmul(psb[:], mixT[:], b2p_t[:], start=True, stop=True)

                acc = fin.tile([B, OUT_W], f32)
                nc.vector.tensor_scalar_mul(acc[:], ps2s[0][:], mixpair[:, 0:1])
                acc2 = fin.tile([B, OUT_W], f32)
                nc.vector.scalar_tensor_tensor(
                    acc2[:], ps2s[1][:], mixpair[:, 1:2], acc[:], op0=MULT, op1=ADD
                )
                osb = fin.tile([B, OUT_W], f32)
                nc.vector.tensor_add(osb[:], acc2[:], psb[:])
                nc.sync.dma_start(out, osb[:])

    nc.compile()
    return nc


def pack_x(x):
    """Build the shared (core-independent) prepacked input layouts."""
    x = np.ascontiguousarray(np.asarray(x, np.float32))
    xpad = np.zeros((CIN, B, HPAD, HPAD), np.float32)
    xpad[:, :, 2:66, 2:66] = x.transpose(1, 0, 2, 3)
    xpadf = np.zeros((CIN, XPADF), np.float32)
    xpadf[:, : B * XPLANE] = xpad.reshape(CIN, -1)
    sz = 4
    base = np.lib.stride_tricks.as_strided(
        xpadf, shape=(CIN, 5, 5, B, IMW),
        strides=(XPADF * sz, HPAD * sz, sz, XPLANE * sz, sz),
    )
    imcd = np.ascontiguousarray(base).reshape(75, B, IMW)
    basex = np.lib.stride_tricks.as_strided(
        xpadf, shape=(CIN, 36, B // 2, 2, 2, HPAD),
        strides=(XPADF * sz, HPAD * sz, 2 * XPLANE * sz, XPLANE * sz,
                 32 * HPAD * sz, sz),
    )
    xrd = np.ascontiguousarray(basex).reshape(108, B // 2, 4, HPAD)
    return imcd, xrd


def host_pack(inputs, core):
    node_weights = np.asarray(inputs["node_weights"], np.float32)
    node_biases = np.asarray(inputs["node_biases"], np.float32)
    cw1s = np.asarray(inputs["cw1s"], np.float32)
    cw2s = np.asarray(inputs["cw2s"], np.float32)
    w1s = np.asarray(inputs["w1s"], np.float32)
    b1s = np.asarray(inputs["b1s"], np.float32)
    w2s = np.asarray(inputs["w2s"], np.float32)
    b2s = np.asarray(inputs["b2s"], np.float32)

    l0 = 2 * core

    # conv1 lhsT (75, 128): row p=(c,dy,dx), col m=(leaf, ch)
    w1T = np.zeros((75, 128), np.float32)
    for leaf in range(2):
        w1T[:, 64 * leaf : 64 * leaf + 64] = (
            cw1s[l0 + leaf].transpose(1, 2, 3, 0).reshape(75, 64)
        )

    # routing lhsT (108, 5, 128): row k=(ch, rr), col m=(f, o); per dx
    nw = node_weights[[0, 2, 6, 14], 0]  # (4, 3, 5, 5)
    rtw = np.zeros((108, 5, 128), np.float32)
    ovec = np.arange(32)
    for f in range(4):
        for ch in range(CIN):
            for dy in range(5):
                rtw[36 * ch + ovec + dy, :, 32 * f + ovec] = nw[f, ch, dy, :]

    # conv2 lhsT (128, 2, 3, 5, 128): row k=(r, ci), col m=(o, co);
    # block (r, o) of pass p holds W[:, :, dy=2p+r-o, dx] when 0<=dy<=4
    cw2n = np.zeros((128, 2, 3, 5, 128), np.float32)
    for leaf in range(2):
        w = cw2s[l0 + leaf]  # (co, ci, dy, dx)
        for p in range(3):
            for r in range(2):
                for oo in range(2):
                    dy = 2 * p + r - oo
                    if 0 <= dy <= 4:
                        cw2n[64 * r : 64 * r + 64, leaf, p, :, 64 * oo : 64 * oo + 64] = (
                            w[:, :, dy, :].transpose(1, 2, 0)
                        )

    w1sT = np.stack([w1s[l0], w1s[l0 + 1]], axis=1)  # (64, 2, 128)
    w2sT = np.stack([w2s[l0], w2s[l0 + 1]], axis=1)  # (128, 2, 100)
    b1sT = np.stack([b1s[l0], b1s[l0 + 1]], axis=1)  # (128, 2)
    b2p = np.stack([b2s[l0], b2s[l0 + 1]], axis=0)  # (2, 100)

    rbias = np.zeros((4, 1), np.float32)
    alfv = np.zeros((4, 1), np.float32)
    betv = np.zeros((4, 1), np.float32)
    for d in range(4):
        plat = 2**d - 1
        g = l0 >> (3 - d)
        j, s = g >> 1, g & 1
        rbias[d, 0] = node_biases[plat + j, 0]
        if d < 3:
            alfv[d, 0], betv[d, 0] = (1.0, 0.0) if s == 1 else (-1.0, 1.0)
        else:
            alfv[d, 0], betv[d, 0] = 1.0, 0.0
    return dict(
        w1T=w1T, rtw=rtw, cw2n=cw2n, w1sT=w1sT,
        w2sT=np.ascontiguousarray(w2sT), b1sT=np.ascontiguousarray(b1sT),
        b2p=np.ascontiguousarray(b2p), rbias=rbias, alf=alfv, bet=betv,
        hpz=np.zeros((128, HPROWS, HP), np.float32),
    )


def kernel(**inputs):
    from concourse import bass_utils

    if "nc" not in _cache:
        _cache["nc"] = _build()
    nc = _cache["nc"]
    imcd, xrd = pack_x(inputs["x"])
    in_maps = []
    for c in range(NCORES):
        m = host_pack(inputs, c)
        m["imcd"] = imcd
        m["xrd"] = xrd
        in_maps.append(m)
    res = bass_utils.run_bass_kernel_spmd(nc, in_maps, core_ids=list(range(NCORES)))
    total = np.zeros((B, OUT_W), np.float32)
    for c in range(NCORES):
        total += res.results[c]["out"]
    return total


# revision 30
# speedup vs baseline: 2.0215x; 1.0098x over previous
"""Trainium2 Bass kernel for nn_BaseConvFFF (soft-routed conv mixture-of-experts).

Sharding: expert-parallel — each of the 8 cores computes 2 of the 16 leaves
(full batch), plus the full routing scores; host sums the 8 partial
mixture-weighted outputs.

Per-core device program (per image):
  conv1 (3->64ch, 5x5 SAME): K=75 im2col matmul, M=128 (2 leaves x 64ch),
    8 passes of N=512
  routing convs (4 filters): M=128 = (32 out rows x 4 filters), K=108 =
    (3ch x 36 in rows), 5 dx passes of N=256 per image PAIR
  2x2 maxpool (DVE reduce) + relu eviction into per-leaf planes: lower
    half (Pool engine) plain, upper half (Act engine) shifted +1 ROW
  conv2 (64->64ch, 5x5 SAME): M=128 = (2 out-row parity x 64ch), K=128 =
    (in-row parity x 64ch), 15 passes (3 row-pairs x 5 dx) of N=512 per leaf
  global spatial max (DVE) -> 2-layer MLP -> mixture weighting
"""

import sys

if "/opt/trn_rl_repo" not in sys.path:
    sys.path.append("/opt/trn_rl_repo")

import numpy as np

B, CIN = 32, 3
NCORES = 8
HP = 36  # padded pooled plane cols (32 + 2*2)
HPROWS = 36  # padded pooled plane rows (must be even for the pair view)
HPAD = 68  # padded conv1 input plane (64 + 2*2)
XPLANE = HPAD * HPAD  # 4624
XPADF = B * XPLANE + 64  # flat padded planes per channel + overrun tail
IMW = 64 * HPAD  # 4352: one im2col row (64 rows x 68, contiguous source)
OUT_W = 100

_cache = {}


def _build(opts=None):
    import concourse.bass as bass
    import concourse.tile as tile
    from concourse import bacc, mybir

    f32 = mybir.dt.float32
    f32r = mybir.dt.float32r
    MAX = mybir.AluOpType.max
    MULT = mybir.AluOpType.mult
    ADD = mybir.AluOpType.add
    AX = mybir.AxisListType.X
    AXY = mybir.AxisListType.XY
    RELU = mybir.ActivationFunctionType.Relu

    o = dict(imcol=5, tmpb=3, xrb=3, psc=2, pse=2, psr=2, slots=2,
             lower_eng="gp", upper_eng="act", hiprio=True, dbg=False)
    if opts:
        o.update(opts)
    nc = bacc.Bacc("TRN2", target_bir_lowering=False, debug=False, num_devices=NCORES)

    def din(name, shape, dt):
        return nc.dram_tensor(name, list(shape), dt, kind="ExternalInput").ap()

    imcd = din("imcd", (75, B, IMW), f32r)
    xrd = din("xrd", (108, B // 2, 4, HPAD), f32r)
    w1T = din("w1T", (75, 128), f32r)
    rtw = din("rtw", (108, 5, 128), f32r)
    cw2n = din("cw2n", (128, 2, 3, 5, 128), f32r)
    w1sT = din("w1sT", (64, 2, 128), f32r)
    w2sT = din("w2sT", (128, 2, 100), f32r)
    b1sT = din("b1sT", (128, 2), f32)
    b2p = din("b2p", (2, 100), f32r)
    rbias = din("rbias", (4, 1), f32)
    alf = din("alf", (4, 1), f32)
    bet = din("bet", (4, 1), f32)
    hpz = din("hpz", (128, HPROWS, HP), f32r)
    out = nc.dram_tensor("out", [B, OUT_W], f32, kind="ExternalOutput").ap()
    if o["dbg"]:
        dfeat0 = nc.dram_tensor("dfeat0", [128, B], f32, kind="ExternalOutput").ap()
        dfeat1 = nc.dram_tensor("dfeat1", [128, B], f32, kind="ExternalOutput").ap()
        drtsc = nc.dram_tensor("drtsc", [128, B], f32, kind="ExternalOutput").ap()
        dhp00 = nc.dram_tensor("dhp00", [128, HPROWS, HP], f32r, kind="ExternalOutput").ap()
        dhp10 = nc.dram_tensor("dhp10", [128, HPROWS, HP], f32r, kind="ExternalOutput").ap()

    with tile.TileContext(nc) as tc:
        with (
            tc.tile_pool(name="const", bufs=1) as cp,
            tc.tile_pool(name="pers", bufs=1) as pers,
        ):
            # critical consts first (needed by image 0's conv1)
            w1T_t = cp.tile([75, 128], f32r)
            nc.sync.dma_start(w1T_t[:], w1T)
            # everything else: allocate now, DMA after image 0's input DMAs
            rtw_t = cp.tile([108, 5, 128], f32r)
            cw2n_t = cp.tile([128, 2, 3, 5, 128], f32r)
            w1sT_t = cp.tile([64, 2, 128], f32r)
            w2sT_t = cp.tile([128, 2, 100], f32r)
            b1sT_t = cp.tile([128, 2], f32)
            b2p_t = cp.tile([2, 100], f32r)
            rbias_t = cp.tile([4, 1], f32)
            alf_t = cp.tile([4, 1], f32)
            bet_t = cp.tile([4, 1], f32)

            def emit_late_consts():
                nc.sync.dma_start(rtw_t[:], rtw)
                nc.sync.dma_start(cw2n_t[:, 0], cw2n[:, 0])
                nc.sync.dma_start(cw2n_t[:, 1], cw2n[:, 1])
                nc.sync.dma_start(w1sT_t[:], w1sT)
                nc.sync.dma_start(w2sT_t[:], w2sT)
                nc.sync.dma_start(b1sT_t[:], b1sT)
                nc.sync.dma_start(b2p_t[:], b2p)
                nc.sync.dma_start(rbias_t[:], rbias)
                nc.sync.dma_start(alf_t[:], alf)
                nc.sync.dma_start(bet_t[:], bet)

            # persistent working buffers
            # hp tiles (one per leaf x slot): partitions 0:64 = pooled plane
            # (relu'd), partitions 64:128 = same plane shifted UP one row
            # (for conv2 dy pairing). Zeroed on-device (pads stay zero).
            hps = [
                [pers.tile([128, HPROWS, HP], f32r, name=f"hp{l}_{s}")
                 for s in range(o["slots"])]
                for l in range(2)
            ]
            featsc0 = pers.tile([128, B], f32)
            featsc1 = pers.tile([128, B], f32)
            featscs = (featsc0, featsc1)
            rtsc = pers.tile([128, B], f32)

            with (
                tc.tile_pool(name="imcol", bufs=o["imcol"]) as impool,
                tc.tile_pool(name="xr", bufs=o["xrb"]) as xrpool,
                tc.tile_pool(name="tmp", bufs=o["tmpb"]) as tmppool,
                tc.tile_pool(name="psc", bufs=o["psc"], space="PSUM") as pscp,
                tc.tile_pool(name="pse", bufs=o["pse"], space="PSUM") as psep,
                tc.tile_pool(name="psr", bufs=o["psr"], space="PSUM") as psrp,
            ):
                def conv2_stream(b):
                    """Yields after each conv2 matmul so the caller can
                    interleave chunks between conv1 pairs of the next image."""
                    slot = b % o["slots"]
                    for leaf in range(2):
                        hpv = hps[leaf][slot].rearrange(
                            "p (r t) c -> p r t c", t=2
                        )
                        psE = psep.tile([128, 16, 32], f32, name="psE")
                        k = 0
                        for p in range(3):
                            for dx in range(5):
                                k += 1
                                nc.tensor.matmul(
                                    psE[:],
                                    cw2n_t[:, leaf, p, dx, :],
                                    hpv[:, p : p + 16, 0, dx : dx + 32],
                                    start=(k == 1), stop=(k == 15),
                                )
                                yield
                        nc.vector.tensor_reduce(
                            featscs[leaf][:, b : b + 1],
                            psE.rearrange("p q x -> p (q x)"),
                            axis=AX, op=MAX,
                        )

                def pull(gen, n):
                    if gen is not None:
                        for _ in range(n):
                            if next(gen, "done") == "done":
                                break

                gen = None
                for b in range(B):
                    slot = b % o["slots"]
                    # ---- im2col: partition p=(c,dy,dx) holds the padded
                    # plane shifted by (dy,dx) — contiguous 4352-elem source
                    imc = impool.tile([75, IMW], f32r)
                    if b == 0:
                        # split so conv1 pair0/1 can start after half the DMA
                        nc.sync.dma_start(imc[:, 0 : IMW // 2], imcd[:, b, 0 : IMW // 2])
                        nc.sync.dma_start(imc[:, IMW // 2 :], imcd[:, b, IMW // 2 :])
                    else:
                        nc.sync.dma_start(imc[:], imcd[:, b, :])
                    imcv = imc.rearrange("p (y x) -> p y x", y=64, x=HPAD)

                    # ---- routing input rows (image pair): partition
                    # (ch, rr) holds padded rows 32h+rr of images b, b+1
                    if b % 2 == 0:
                        xr = xrpool.tile([108, 4, HPAD], f32r)
                        nc.sync.dma_start(xr[:], xrd[:, b // 2, :, :])
                    if b == 0:
                        # hp zero-init for slot 0 first (needed by image 0's
                        # evictions), then weights, then slot 1
                        nc.sync.dma_start(hps[0][0][:], hpz)
                        nc.sync.dma_start(hps[1][0][:], hpz)
                        emit_late_consts()
                        nc.sync.dma_start(hps[0][1][:], hpz)
                        nc.sync.dma_start(hps[1][1][:], hpz)

    # ---- conv1: 128 out-ch (2 leaves), K=75 matmuls into a
                    # 2-bank psum tile (two 512-px halves); conv2 chunks of
                    # the previous image fill PE while reduces/evicts run.
                    # The conv1->reduce->evict chain is emitted at high
                    # priority so the scheduler starts it as soon as inputs
                    # land; conv2 of the previous image then fills the PE
                    # while the chain completes.
                    import contextlib
                    hctx = tc.high_priority if o["hiprio"] else contextlib.nullcontext
                    for pair in range(4):
                      with hctx():
                        psC2 = pscp.tile([128, 2, 512], f32)
                        for h in range(2):
                            t = 2 * pair + h
                            rhs = imcv[:, 8 * t : 8 * t + 8, 0:64]
                            nc.tensor.matmul(
                                psC2[:, h, :], w1T_t[:], rhs,
                                start=True, stop=True,
                            )
                        # maxpool 2x2 via one 5D reduce (single PSUM input —
                        # the HW verifier allows only one PSUM read per op),
                        # then gpsimd relu-evict lower + Act relu-evict the
                        # +1-row shifted upper block.
                        pcv = psC2.rearrange(
                            "p h (yb wy x wx) -> p (h yb) x wy wx",
                            yb=4, wy=2, x=32, wx=2,
                        )
                        tx = tmppool.tile([128, 8, 32], f32)
                        nc.vector.tensor_reduce(tx[:], pcv[:], axis=AXY, op=MAX)
                        y0 = 2 + 8 * pair
                        for leaf in range(2):
                            hp = hps[leaf][slot]
                            th = tx[64 * leaf : 64 * leaf + 64, :, :]
                            nc.gpsimd.tensor_scalar_max(
                                hp[0:64, y0 : y0 + 8, 2:34], th, 0.0
                            )
                            # upper block: same, shifted one ROW up
                            nc.scalar.activation(
                                hp[64:128, y0 - 1 : y0 + 7, 2:34], th, RELU
                            )
                      if pair >= 1:
                          pull(gen, 8)

                    # ---- routing matmuls: M=(f,o), K=(ch,rr), accumulate
                    # over 5 dx passes; then max over (h, x) per image
                    if b % 2 == 0:
                        # full-bank tile: matmul start=True resets the whole
                        # 2KB PSUM bank, so psR must not share a bank
                        psR = psrp.tile([128, 2, 2, 128], f32, name="psR")
                        for dx in range(5):
                            nc.tensor.matmul(
                                psR[:, 0], rtw_t[:, dx, :], xr[:, :, dx : dx + 64],
                                start=(dx == 0), stop=(dx == 4),
                            )
                        nc.vector.tensor_reduce(
                            rtsc[:, b : b + 2], psR[:, 0], axis=AX, op=MAX,
                        )
                    pull(gen, 99)
                    gen = conv2_stream(b)
                pull(gen, 99)

            # ---------------- finalize: routing mix + MLP ----------------
            with (
                tc.tile_pool(name="fin", bufs=1) as fin,
                tc.tile_pool(name="psm", bufs=1, space="PSUM") as psm,
            ):
                if o["dbg"]:
                    nc.sync.dma_start(dfeat0, featsc0[:])
                    nc.sync.dma_start(dfeat1, featsc1[:])
                    nc.sync.dma_start(drtsc, rtsc[:])
                    nc.sync.dma_start(dhp00, hps[0][0][:])
                    nc.sync.dma_start(dhp10, hps[1][0][:])
                rtj = fin.tile([4, 32, B], f32)
                nc.sync.dma_start(rtj[:], rtsc[:])
                scoresT = fin.tile([4, B], f32)
                nc.vector.reduce_max(
                    scoresT[:], rtj.rearrange("d j b -> d b j"), axis=AX
                )
                sg = fin.tile([4, B], f32)
                nc.scalar.activation(
                    sg[:], scoresT[:], mybir.ActivationFunctionType.Sigmoid,
                    bias=rbias_t[:, 0:1],
                )
                fsel = fin.tile([4, B], f32)
                nc.vector.tensor_scalar(
                    fsel[:], sg[:], alf_t[:, 0:1], bet_t[:, 0:1], op0=MULT, op1=ADD
                )
                fT = fin.tile([B, 4], f32)
                for d in range(4):
                    nc.sync.dma_start(fT[:, d : d + 1], fsel[d : d + 1, :])
                t01 = fin.tile([B, 1], f32)
                nc.vector.tensor_mul(t01[:], fT[:, 0:1], fT[:, 1:2])
                m012 = fin.tile([B, 1], f32)
                nc.vector.tensor_mul(m012[:], t01[:], fT[:, 2:3])
                mixpair = fin.tile([B, 2], f32)
                nc.vector.tensor_mul(mixpair[:, 1:2], m012[:], fT[:, 3:4])
                nc.vector.tensor_sub(mixpair[:, 0:1], m012[:], mixpair[:, 1:2])
                mixpR = fin.tile([B, 2], f32r)
                nc.vector.tensor_copy(mixpR[:], mixpair[:])
                mixT = fin.tile([2, B], f32r)
                for leaf in range(2):
                    nc.sync.dma_start(
                        mixT[leaf : leaf + 1, :], mixpR[:, leaf : leaf + 1]
                    )

                ps2s = []
                for leaf in range(2):
                    # fold the two out-row-parity halves of feat, with relu
                    fu = fin.tile([64, B], f32, name=f"fu{leaf}")
                    nc.sync.dma_start(fu[:], featscs[leaf][64:128, :])
                    featT = fin.tile([64, B], f32r, name=f"featT{leaf}")
                    nc.vector.scalar_tensor_tensor(
                        featT[:], featscs[leaf][0:64, :], 0.0, fu[:],
                        op0=MAX, op1=MAX,
                    )
                    ps1 = psm.tile([128, B], f32, name=f"ps1_{leaf}")
                    nc.tensor.matmul(
                        ps1[:], w1sT_t[:, leaf, :], featT[:], start=True, stop=True
                    )
                    h1b = fin.tile([128, B], f32r, name=f"h1b{leaf}")
                    nc.vector.tensor_scalar_add(
                        h1b[:], ps1[:], b1sT_t[:, leaf : leaf + 1]
                    )
                    ps2 = psm.tile([B, OUT_W], f32, name=f"ps2_{leaf}")
                    nc.tensor.matmul(
                        ps2[:], h1b[:], w2sT_t[:, leaf, :], start=True, stop=True
                    )
                    ps2s.append(ps2)

                psb = psm.tile([B, OUT_W], f32)
                nc.tensor.matmul(psb[:], mixT[:], b2p_t[:], start=True, stop=True)

                acc = fin.tile([B, OUT_W], f32)
                nc.vector.tensor_scalar_mul(acc[:], ps2s[0][:], mixpair[:, 0:1])
                acc2 = fin.tile([B, OUT_W], f32)
                nc.vector.scalar_tensor_tensor(
                    acc2[:], ps2s[1][:], mixpair[:, 1:2], acc[:], op0=MULT, op1=ADD
                )
                osb = fin.tile([B, OUT_W], f32)
                nc.vector.tensor_add(osb[:], acc2[:], psb[:])
                nc.sync.dma_start(out, osb[:])

    nc.compile()
    return nc


def pack_x(x):
    """Build the shared (core-independent) prepacked input layouts."""
    x = np.ascontiguousarray(np.asarray(x, np.float32))
    xpad = np.zeros((CIN, B, HPAD, HPAD), np.float32)
    xpad[:, :, 2:66, 2:66] = x.transpose(1, 0, 2, 3)
    xpadf = np.zeros((CIN, XPADF), np.float32)
    xpadf[:, : B * XPLANE] = xpad.reshape(CIN, -1)
    sz = 4
    base = np.lib.stride_tricks.as_strided(
        xpadf, shape=(CIN, 5, 5, B, IMW),
        strides=(XPADF * sz, HPAD * sz, sz, XPLANE * sz, sz),
    )
    imcd = np.ascontiguousarray(base).reshape(75, B, IMW)
    basex = np.lib.stride_tricks.as_strided(
        xpadf, shape=(CIN, 36, B // 2, 2, 2, HPAD),
        strides=(XPADF * sz, HPAD * sz, 2 * XPLANE * sz, XPLANE * sz,
                 32 * HPAD * sz, sz),
    )
    xrd = np.ascontiguousarray(basex).reshape(108, B // 2, 4, HPAD)
    return imcd, xrd


def host_pack(inputs, core):
    node_weights = np.asarray(inputs["node_weights"], np.float32)
    node_biases = np.asarray(inputs["node_biases"], np.float32)
    cw1s = np.asarray(inputs["cw1s"], np.float32)
    cw2s = np.asarray(inputs["cw2s"], np.float32)
    w1s = np.asarray(inputs["w1s"], np.float32)
    b1s = np.asarray(inputs["b1s"], np.float32)
    w2s = np.asarray(inputs["w2s"], np.float32)
    b2s = np.asarray(inputs["b2s"], np.float32)

    l0 = 2 * core

    # conv1 lhsT (75, 128): row p=(c,dy,dx), col m=(leaf, ch)
    w1T = np.zeros((75, 128), np.float32)
    for leaf in range(2):
        w1T[:, 64 * leaf : 64 * leaf + 64] = (
            cw1s[l0 + leaf].transpose(1, 2, 3, 0).reshape(75, 64)
        )

    # routing lhsT (108, 5, 128): row k=(ch, rr), col m=(f, o); per dx
    nw = node_weights[[0, 2, 6, 14], 0]  # (4, 3, 5, 5)
    rtw = np.zeros((108, 5, 128), np.float32)
    ovec = np.arange(32)
    for f in range(4):
        for ch in range(CIN):
            for dy in range(5):
                rtw[36 * ch + ovec + dy, :, 32 * f + ovec] = nw[f, ch, dy, :]

    # conv2 lhsT (128, 2, 3, 5, 128): row k=(r, ci), col m=(o, co);
    # block (r, o) of pass p holds W[:, :, dy=2p+r-o, dx] when 0<=dy<=4
    cw2n = np.zeros((128, 2, 3, 5, 128), np.float32)
    for leaf in range(2):
        w = cw2s[l0 + leaf]  # (co, ci, dy, dx)
        for p in range(3):
            for r in range(2):
                for oo in range(2):
                    dy = 2 * p + r - oo
                    if 0 <= dy <= 4:
                        cw2n[64 * r : 64 * r + 64, leaf, p, :, 64 * oo : 64 * oo + 64] = (
                            w[:, :, dy, :].transpose(1, 2, 0)
                        )

    w1sT = np.stack([w1s[l0], w1s[l0 + 1]], axis=1)  # (64, 2, 128)
    w2sT = np.stack([w2s[l0], w2s[l0 + 1]], axis=1)  # (128, 2, 100)
    b1sT = np.stack([b1s[l0], b1s[l0 + 1]], axis=1)  # (128, 2)
    b2p = np.stack([b2s[l0], b2s[l0 + 1]], axis=0)  # (2, 100)

    rbias = np.zeros((4, 1), np.float32)
    alfv = np.zeros((4, 1), np.float32)
    betv = np.zeros((4, 1), np.float32)
    for d in range(4):
        plat = 2**d - 1
        g = l0 >> (3 - d)
        j, s = g >> 1, g & 1
        rbias[d, 0] = node_biases[plat + j, 0]
        if d < 3:
            alfv[d, 0], betv[d, 0] = (1.0, 0.0) if s == 1 else (-1.0, 1.0)
        else:
            alfv[d, 0], betv[d, 0] = 1.0, 0.0
    return dict(
        w1T=w1T, rtw=rtw, cw2n=cw2n, w1sT=w1sT,
        w2sT=np.ascontiguousarray(w2sT), b1sT=np.ascontiguousarray(b1sT),
        b2p=np.ascontiguousarray(b2p), rbias=rbias, alf=alfv, bet=betv,
        hpz=np.zeros((128, HPROWS, HP), np.float32),
    )


def kernel(**inputs):
    from concourse import bass_utils

    if "nc" not in _cache:
        _cache["nc"] = _build()
    nc = _cache["nc"]
    imcd, xrd = pack_x(inputs["x"])
    in_maps = []
    for c in range(NCORES):
        m = host_pack(inputs, c)
        m["imcd"] = imcd
        m["xrd"] = xrd
        in_maps.append(m)
    res = bass_utils.run_bass_kernel_spmd(nc, in_maps, core_ids=list(range(NCORES)))
    total = np.zeros((B, OUT_W), np.float32)
    for c in range(NCORES):
        total += res.results[c]["out"]
    return total
